# revision 1
# baseline (speedup 1.0000x reference)
"""Trainium2 Bass kernel for nn_DPRNN_TAC (DPRNN + TAC, L=2 layers).

Sharding: one (batch, channel) pair per NeuronCore (B*CH = 8 = n_cores).
Row/col BiLSTMs, group norms and TAC MLPs are core-local; the TAC channel
mean is a ReduceScatter + AllGather over the 4 cores of each batch group.

Math tricks (validated against the fp32 reference in a numpy golden model):
 - All LSTM gate nonlinearities are evaluated with tanh only
   (sigmoid(x) = 0.5 + 0.5*tanh(x/2)); the 0.5 input scale for gates
   i, f, o is folded into the weights host-side.
 - Stored hidden state is 2h; the 0.5 correction is folded into Whh and
   the projection weights host-side.
 - The masked channel mean divides by eff = num_mic[b]; 1/eff is folded
   into av_w host-side, and cores with channel >= eff contribute zeros.
Precision: scan weights/activations bf16 (fp32 psum + fp32 cell state),
big MLPs float32r, group-norm statistics fp32.

Memory: the residual stream ("out") and the BiLSTM hidden histories live
in HBM and are streamed in chunks; SBUF holds the scan working set, the
current x-cast (bf16) and the pre-norm z tensor.
"""

import numpy as np
import ml_dtypes

BF16 = ml_dtypes.bfloat16

L, B, CH, N, H, D1, D2, OUT = 2, 2, 4, 64, 128, 100, 200, 64
E = 3 * H
NCORES = 8

_CACHE = {}


def prep_weights(inp, core_b, core_c, layers=L):
    """Host-side weight transforms for one core. Returns dict[str, np.ndarray]."""
    gs = np.array([0.5, 0.5, 1.0, 0.5], np.float32)  # gate scales i,f,g,o
    num_mic = np.asarray(inp["num_mic"]).astype(np.int64)
    eff = int(num_mic[core_b]) if int(num_mic.max()) > 0 else CH
    if eff <= 0:
        eff = CH
    w = {}
    scan_bias_nonzero = False
    for r in ("row", "col"):
        for i in range(layers):
            bsum = np.asarray(inp[f"{r}_bih"][i]) + np.asarray(inp[f"{r}_bhh"][i])
            if np.any(np.asarray(bsum) != 0):
                scan_bias_nonzero = True
    w["_has_xbias"] = scan_bias_nonzero
    KX = 65 if scan_bias_nonzero else 64

    for r in ("row", "col"):
        for i in range(layers):
            whh = np.zeros((2, 4, H, H), np.float32)
            wxb = np.zeros((2, 4, KX, H), np.float32)
            for d in range(2):
                Wih = np.asarray(inp[f"{r}_Wih"][i][d], np.float32).reshape(4, H, N)
                Whh = np.asarray(inp[f"{r}_Whh"][i][d], np.float32).reshape(4, H, H)
                bsum = (
                    np.asarray(inp[f"{r}_bih"][i][d], np.float32)
                    + np.asarray(inp[f"{r}_bhh"][i][d], np.float32)
                ).reshape(4, H)
                for g in range(4):
                    whh[d, g] = (Whh[g] * gs[g] * 0.5).T  # lhsT [h, gh]
                    wx = (Wih[g] * gs[g]).T  # [N, H]
                    if scan_bias_nonzero:
                        wxb[d, g] = np.vstack([wx, (bsum[g] * gs[g])[None, :]])
                    else:
                        wxb[d, g] = wx
            # SBUF layout: partition dim first
            w[f"whh_{r}{i}"] = np.moveaxis(whh, 2, 0).astype(BF16)  # [H,2,4,H]
            w[f"wxb_{r}{i}"] = np.moveaxis(wxb, 2, 0).astype(BF16)  # [KX,2,4,H]
            pw = np.asarray(inp[f"{r}_proj_w"][i], np.float32)  # [N, 2H]
            pj = np.zeros((2, H, N), np.float32)
            pj[0] = (0.5 * pw[:, :H]).T
            pj[1] = (0.5 * pw[:, H:]).T
            w[f"proj_{r}{i}"] = np.moveaxis(pj, 1, 0).astype(BF16)  # [H,2,N]
            w[f"pb_{r}{i}"] = np.asarray(
                inp[f"{r}_proj_b"][i], np.float32).reshape(N, 1)
    for i in range(layers):
        trw = np.asarray(inp["tr_w"][i], np.float32)  # [E, N]
        w[f"trw{i}"] = np.ascontiguousarray(trw.T.reshape(N, 3, H))
        w[f"trb{i}"] = np.ascontiguousarray(
            np.asarray(inp["tr_b"][i], np.float32).reshape(3, H).T)  # [H,3]
        avw = np.asarray(inp["av_w"][i], np.float32) / float(eff)  # [E, E]
        avw4 = np.ascontiguousarray(avw.T.reshape(3, H, 3, H))  # [kt,k,mt,m]
        w[f"avw{i}"] = np.moveaxis(avw4, 1, 0).astype(BF16)  # [H,kt,mt,m]
        w[f"avb{i}"] = np.ascontiguousarray(
            np.asarray(inp["av_b"][i], np.float32).reshape(3, H).T)  # [H,3]
        ccw = np.asarray(inp["cc_w"][i], np.float32)  # [N, 2E]
        ccw6 = np.ascontiguousarray(ccw.T.reshape(6, H, N))
        w[f"ccw{i}"] = np.moveaxis(ccw6, 1, 0).astype(BF16)  # [H,6,N]
        w[f"ccb{i}"] = np.asarray(inp["cc_b"][i], np.float32).reshape(N, 1)
        for nm in ("rn", "cn", "chn"):
            w[f"{nm}w{i}"] = np.asarray(inp[f"{nm}_w"][i], np.float32).reshape(N, 1)
            w[f"{nm}b{i}"] = np.asarray(inp[f"{nm}_b"][i], np.float32).reshape(N, 1)
    w["outw"] = np.ascontiguousarray(np.asarray(inp["out_w"], np.float32).T)
    w["outb"] = np.asarray(inp["out_b"], np.float32).reshape(OUT, 1)
    w["msk"] = np.full((H, 1), 1.0 if core_c < eff else 0.0, np.float32)
    return w


def build_program(has_xbias, alphas, bias_flags, d1=D1, d2=D2, layers=L,
                  n_cores=NCORES, n_iter=1, with_cc=True):
    import concourse.bass as bass
    import concourse.tile as tile
    from concourse import bacc, mybir
    import contextlib

    f32 = mybir.dt.float32
    f32r = mybir.dt.float32r
    bf16 = mybir.dt.bfloat16
    AF = mybir.ActivationFunctionType
    OP = mybir.AluOpType

    pos = d1 * d2
    assert pos % 4 == 0
    blk = pos // 4  # allreduce block per group rank
    # chunk width for p-major pointwise loops
    cw = next(c for c in (512, 500, 400, 256, 200, 128, 100, 64, 48, 32, 20, 12, 8, 4)
              if pos % c == 0)
    nch = pos // cw
    # cn-apply chunk: whole p-rows, <=512
    rows_per = max(1, 400 // d2) if d2 <= 400 else 1
    cwc = rows_per * d2
    while pos % cwc != 0:
        rows_per -= 1
        cwc = rows_per * d2
    nchc = pos // cwc
    tr_a, av_a, cc_a, out_a = alphas
    tr_bnz, av_bnz, cc_bnz = bias_flags
    KX = 65 if has_xbias else 64
    n_groups = n_cores // 4
    rgroups = [[g * 4 + j for j in range(4)] for g in range(n_groups)]

    nc = bacc.Bacc("TRN2", target_bir_lowering=False, debug=False,
                   num_devices=n_cores)

    x_in = nc.dram_tensor("x", [N, pos], f32, kind="ExternalInput")
    y_out = nc.dram_tensor("y", [OUT, pos], f32, kind="ExternalOutput")

    def din(name, shape, dt):
        return nc.dram_tensor(name, shape, dt, kind="ExternalInput")

    wt = {}
    for r in ("row", "col"):
        for i in range(layers):
            wt[f"whh_{r}{i}"] = din(f"whh_{r}{i}", [H, 2, 4, H], bf16)
            wt[f"wxb_{r}{i}"] = din(f"wxb_{r}{i}", [KX, 2, 4, H], bf16)
            wt[f"proj_{r}{i}"] = din(f"proj_{r}{i}", [H, 2, N], bf16)
            wt[f"pb_{r}{i}"] = din(f"pb_{r}{i}", [N, 1], f32)
    for i in range(layers):
        wt[f"trw{i}"] = din(f"trw{i}", [N, 3, H], f32r)
        wt[f"trb{i}"] = din(f"trb{i}", [H, 3], f32)
        wt[f"avw{i}"] = din(f"avw{i}", [H, 3, 3, H], bf16)
        wt[f"avb{i}"] = din(f"avb{i}", [H, 3], f32)
        wt[f"ccw{i}"] = din(f"ccw{i}", [H, 6, N], bf16)
        wt[f"ccb{i}"] = din(f"ccb{i}", [N, 1], f32)
        for nm in ("rn", "cn", "chn"):
            wt[f"{nm}w{i}"] = din(f"{nm}w{i}", [N, 1], f32)
            wt[f"{nm}b{i}"] = din(f"{nm}b{i}", [N, 1], f32)
    wt["outw"] = din("outw", [N, OUT], f32r)
    wt["outb"] = din("outb", [OUT, 1], f32)
    wt["msk"] = din("msk", [H, 1], f32)

    with tile.TileContext(nc) as tc:
        with contextlib.ExitStack() as ctx:
            singles = ctx.enter_context(tc.tile_pool(name="singles", bufs=1))
            xz = ctx.enter_context(tc.tile_pool(name="xz", bufs=2))
            ps = ctx.enter_context(tc.tile_pool(name="ps", bufs=2, space="PSUM"))
            tmp = ctx.enter_context(tc.tile_pool(name="tmp", bufs=2))
            chk = ctx.enter_context(tc.tile_pool(name="chk", bufs=2))
            dram = ctx.enter_context(tc.tile_pool(name="dram", bufs=1, space="DRAM"))

            sw = {}
            for k, t in wt.items():
                sw[k] = singles.tile(list(t.shape), t.dtype, tag=f"w_{k}", name=f"sw_{k}")
                nc.sync.dma_start(out=sw[k][:], in_=t[:])

            out_dram = dram.tile([N, pos], f32r)
            ch_dram = dram.tile([H, 3, pos], bf16)
            hf_dram = dram.tile([H, pos], bf16)
            hb_dram = dram.tile([H, pos], bf16)
            bounce_in = dram.tile([4, 3, H, blk], bf16)
            bounce_rs = dram.tile([3, H, blk], bf16)
            bounce_cm = dram.tile([4, 3, H, blk], bf16)
            gn_dram = dram.tile([1, 2], f32)

            def new_x65():
                return xz.tile([KX, pos], bf16, tag="xz", name="x65t")

            def new_z():
                return xz.tile([N, pos], bf16, tag="xz", name="zt")

            ones_k = singles.tile([N, 1], f32, tag="ones_k")
            nc.vector.memset(ones_k[:], 1.0)
            epsc = singles.tile([1, 1], f32, tag="epsc")
            nc.vector.memset(epsc[:], 1e-8)

            def gn_finalize(stats, wv, bv):
                """stats [N, nchunks, 6] -> per-partition s,t [N,1] f32."""
                mv = tmp.tile([N, 2], f32, tag="gnmv")
                nc.vector.bn_aggr(out=mv[:], in_=stats[:])
                r3 = tmp.tile([N, 3], f32, tag="gnr3")
                nc.vector.tensor_copy(r3[:, 0:2], mv[:, 0:2])
                nc.vector.tensor_mul(r3[:, 2:3], mv[:, 0:1], mv[:, 0:1])
                pr = ps.tile([128, 4, 512], f32, tag="ps")
                nc.tensor.matmul(pr[0:1, 0, 0:3], ones_k[:], r3[:],
                                 start=True, stop=True)
                sc = tmp.tile([1, 3], f32, tag="gnsc")
                nc.vector.tensor_scalar_mul(sc[0:1, 0:3], pr[0:1, 0, 0:3], 1.0 / N)
                var = tmp.tile([1, 1], f32, tag="gnvar")
                nc.vector.tensor_mul(var[0:1], sc[0:1, 0:1], sc[0:1, 0:1])
                nc.vector.tensor_sub(var[0:1], sc[0:1, 1:2], var[0:1])
                nc.vector.tensor_add(var[0:1], var[0:1], sc[0:1, 2:3])
                # wait: var = S1/N - mu^2 + S2/N  (S1=sum var_p, S2=sum mu_p^2)
                rm = tmp.tile([1, 2], f32, tag="gnrm")
                nc.scalar.activation(out=rm[0:1, 0:1], in_=var[0:1], func=AF.Sqrt,
                                     bias=epsc[0:1, :], scale=1.0)
                nc.vector.reciprocal(rm[0:1, 0:1], rm[0:1, 0:1])
                nc.vector.tensor_scalar_mul(rm[0:1, 1:2], sc[0:1, 0:1], -1.0)
                nc.sync.dma_start(out=gn_dram[0:1, :], in_=rm[0:1, :])
                rb = tmp.tile([N, 2], f32, tag="gnrb")
                bcast = bass.AP(tensor=gn_dram[:].tensor, offset=gn_dram[:].offset,
                                ap=[[0, N], [1, 2]])
                nc.sync.dma_start(out=rb[:], in_=bcast)
                s = tmp.tile([N, 1], f32, tag="gns")
                t = tmp.tile([N, 1], f32, tag="gnt")
                nc.vector.tensor_mul(s[:], wv[:], rb[:, 0:1])
                nc.vector.scalar_tensor_tensor(
                    out=t[:], in0=s[:], scalar=rb[:, 1:2], in1=bv[:],
                    op0=OP.mult, op1=OP.add)
                return s, t

            # ---------------- LSTM scan ----------------
            def scan(r, i, x65, T, M, x_ap, hdst, seq_base):
                """Interleaved fwd/bwd scan. x_ap(d, k) -> AP [KX, M];
                hdst(d, seq) -> DRAM AP [H, M]; cell state in SBUF ring."""
                whh = sw[f"whh_{r}{i}"]
                wxb = sw[f"wxb_{r}{i}"]
                wst = tmp.tile([H, 2, M], f32, tag="wst")
                nc.vector.memset(wst[:], 0.0)
                hr = tmp.tile([H, 2, 2, M], bf16, tag="hr")  # [h, k%2, dir, M]
                for k in range(T):
                    pst = ps.tile([128, 4, 512], f32, tag="ps")
                    for d in range(2):
                        sl = slice(d * M, (d + 1) * M)
                        for g in range(4):
                            if k > 0:
                                nc.tensor.matmul(
                                    pst[:, g, sl], whh[:, d, g, :],
                                    hr[:, (k - 1) % 2, d, :],
                                    start=True, stop=False)
                            nc.tensor.matmul(
                                pst[:, g, sl], wxb[:, d, g, :], x_ap(d, k),
                                start=(k == 0), stop=True)
                    Tt = tmp.tile([H, 4, 2 * M], f32, tag="Tt")
                    nc.scalar.activation(out=Tt[:], in_=pst[:, :, 0:2 * M],
                                         func=AF.Tanh)
                    for d in range(2):
                        sl = slice(d * M, (d + 1) * M)
                        pt = tmp.tile([H, M], f32, tag="pt")
                        qt = tmp.tile([H, M], f32, tag="qt")
                        nc.vector.scalar_tensor_tensor(
                            out=pt[:], in0=Tt[:, 1, sl], scalar=1.0,
                            in1=wst[:, d, :], op0=OP.add, op1=OP.mult)
                        nc.vector.scalar_tensor_tensor(
                            out=qt[:], in0=Tt[:, 0, sl], scalar=1.0,
                            in1=Tt[:, 2, sl], op0=OP.add, op1=OP.mult)
                        nc.vector.scalar_tensor_tensor(
                            out=wst[:, d, :], in0=pt[:], scalar=0.5,
                            in1=qt[:], op0=OP.mult, op1=OP.add)
                    Tc = tmp.tile([H, 2, M], f32, tag="Tc")
                    nc.scalar.activation(out=Tc[:], in_=wst[:], func=AF.Tanh,
                                         scale=0.5)
                    for d in range(2):
                        sl = slice(d * M, (d + 1) * M)
                        seq = k if d == 0 else (T - 1 - k)
                        nc.vector.scalar_tensor_tensor(
                            out=hr[:, k % 2, d, :], in0=Tt[:, 3, sl],
                            scalar=1.0, in1=Tc[:, d, :], op0=OP.add, op1=OP.mult)
                        nc.sync.dma_start(out=hdst(d, seq), in_=hr[:, k % 2, d, :])

            # ------------- proj + GN stats -------------
            def proj_phase(r, i, nchunks, cwp, gnw, gnb):
                """Ĥ streamed from HBM in contiguous cwp chunks."""
                z = new_z()
                pj = sw[f"proj_{r}{i}"]
                pb = sw[f"pb_{r}{i}"]
                stats = tmp.tile([N, nchunks, 6], f32, tag="gnstats")
                for ci in range(nchunks):
                    off = ci * cwp
                    hfc = chk.tile([H, cwp], bf16, tag="hfc")
                    hbc = chk.tile([H, cwp], bf16, tag="hbc")
                    nc.sync.dma_start(out=hfc[:], in_=hf_dram[:, off:off + cwp])
                    nc.sync.dma_start(out=hbc[:], in_=hb_dram[:, off:off + cwp])
                    pp = ps.tile([128, 4, 512], f32, tag="ps")
                    nc.tensor.matmul(pp[0:N, 0, 0:cwp], pj[:, 0, :], hfc[:],
                                     start=True, stop=False)
                    nc.tensor.matmul(pp[0:N, 0, 0:cwp], pj[:, 1, :], hbc[:],
                                     start=False, stop=True)
                    nc.vector.tensor_scalar_add(z[:, off:off + cwp],
                                                pp[0:N, 0, 0:cwp], pb[:])
                    nc.vector.bn_stats(out=stats[:, ci, :], in_=z[:, off:off + cwp])
                s, t = gn_finalize(stats, gnw, gnb)
                return z, s, t

            # ---- chunked apply: out' = out + z*s + t, fused consumer ----
            def apply_chunks(z, s, t, consumer, cwa, ncha, z_ap=None,
                             first_src=None):
                for ci in range(ncha):
                    off = ci * cwa
                    outc = chk.tile([N, cwa], f32r, tag="outc")
                    src = first_src if first_src is not None else out_dram
                    nc.sync.dma_start(out=outc[:],
                                      in_=src[:, off:off + cwa].bitcast(f32r))
                    if z is not None:
                        v = chk.tile([N, cwa], bf16, tag="vt")
                        zin = z_ap(z, ci) if z_ap is not None else z[:, off:off + cwa]
                        nc.vector.tensor_scalar(
                            out=v[:], in0=zin, scalar1=s[:], scalar2=t[:],
                            op0=OP.mult, op1=OP.add)
                        nc.vector.tensor_add(outc[:], outc[:], v[:])
                        nc.sync.dma_start(out=out_dram[:, off:off + cwa],
                                          in_=outc[:])
                    consumer(ci, off, outc)

            def cast_consumer(x65t):
                def f(ci, off, outc):
                    nc.gpsimd.tensor_copy(out=x65t[0:N, off:off + outc.shape[1]],
                                          in_=outc[:])
                return f

            def dma_split_blk(dst4, sb, off, width, write=True):
                """DMA [H,3,width] SBUF <-> bounce[4,3,H,blk], splitting at
                block boundaries."""
                done = 0
                while done < width:
                    o = off + done
                    q, qo = o // blk, o % blk
                    wseg = min(width - done, blk - qo)
                    dr = dst4[q, :, :, qo:qo + wseg].transpose([1, 0, 2])
                    if write:
                        nc.sync.dma_start(out=dr, in_=sb[:, :, done:done + wseg])
                    else:
                        nc.sync.dma_start(out=sb[:, :, done:done + wseg], in_=dr)
                    done += wseg

            # ================= network =================
            def _network():
                x65 = new_x65()
                if has_xbias:
                    nc.vector.memset(x65[N:KX, :], 1.0)

                def init_consumer(x65t):
                    def f(ci, off, outc):
                        nc.sync.dma_start(out=out_dram[:, off:off + cw], in_=outc[:])
                        nc.gpsimd.tensor_copy(out=x65t[0:N, off:off + cw], in_=outc[:])
                    return f

                apply_chunks(None, None, None, init_consumer(x65), cw, nch,
                             first_src=x_in)

                for i in range(layers):
                    # ---- row: sequences along d1 (t=p), batch d2 ----
                    scan("row", i, x65, d1, d2,
                         lambda d, k: x65[0:KX, (k if d == 0 else d1 - 1 - k) * d2:
                                          (k + 1 if d == 0 else d1 - k) * d2],
                         lambda d, seq: (hf_dram if d == 0 else hb_dram)[
                             :, seq * d2:(seq + 1) * d2],
                         0)
                    z, s, t = proj_phase("row", i, nch, cw, sw[f"rnw{i}"], sw[f"rnb{i}"])
                    x65c = new_x65()
                    if has_xbias:
                        nc.vector.memset(x65c[N:KX, :], 1.0)
                    apply_chunks(z, s, t, cast_consumer(x65c), cw, nch)

                    # ---- col: sequences along d2 (t=q), batch d1; strided x ----
                    def xcol(d, k, _x=x65c):
                        q = k if d == 0 else d2 - 1 - k
                        a = _x[0:KX, q:q + 1]
                        return bass.AP(tensor=a.tensor, offset=a.offset,
                                       ap=[a.ap[0], [d2, d1]])

                    scan("col", i, x65c, d2, d1, xcol,
                         lambda d, seq: (hf_dram if d == 0 else hb_dram)[
                             :, seq * d1:(seq + 1) * d1],
                         0)
                    # col proj in q-major contiguous chunks; z stored q-major
                    zq, s2_, t2_ = proj_phase("col", i, nch, cw,
                                              sw[f"cnw{i}"], sw[f"cnb{i}"])

                    # cn-apply in p-row-aligned chunks with strided z view,
                    # fused with TAC tr + mask + bounce write
                    def zq_ap(zt, ci, _rp=rows_per):
                        p0 = ci * _rp
                        a = zt[:, p0:p0 + 1]
                        return bass.AP(tensor=a.tensor, offset=a.offset,
                                       ap=[a.ap[0], [1, _rp], [d1, d2]])

                    trw = sw[f"trw{i}"]
                    trb = sw[f"trb{i}"]
                    msk = sw["msk"]

                    def tr_consumer(ci, off, outc, _i=i):
                        wdt = outc.shape[1]
                        chc = chk.tile([H, 3, cwc], bf16, tag="chc")
                        for e in range(3):
                            pp = ps.tile([128, 4, 512], f32, tag="ps")
                            nc.tensor.matmul(
                                pp[:, 0, 0:wdt], trw[:, e, :],
                                outc[:], start=True, stop=True)
                            if tr_bnz:
                                nc.vector.tensor_scalar_add(
                                    pp[:, 0, 0:wdt], pp[:, 0, 0:wdt],
                                    trb[:, e:e + 1])
                            rl = chk.tile([H, cwc], f32, tag="rl")
                            nc.scalar.activation(
                                out=rl[:, 0:wdt], in_=pp[:, 0, 0:wdt],
                                func=AF.Relu, scale=1.0 - tr_a[_i])
                            nc.vector.scalar_tensor_tensor(
                                out=chc[:, e, 0:wdt], in0=pp[:, 0, 0:wdt],
                                scalar=tr_a[_i], in1=rl[:, 0:wdt],
                                op0=OP.mult, op1=OP.add)
                        nc.sync.dma_start(out=ch_dram[:, :, off:off + wdt],
                                          in_=chc[:, :, 0:wdt])
                        con = chk.tile([H, 3, cwc], bf16, tag="conc")
                        nc.vector.tensor_scalar_mul(con[:, :, 0:wdt],
                                                    chc[:, :, 0:wdt], msk[:])
                        dma_split_blk(bounce_in, con[:, :, 0:wdt], off, wdt)

                    apply_chunks(zq, s2_, t2_, tr_consumer, cwc, nchc, z_ap=zq_ap)

                    if with_cc:
                        nc.gpsimd.collective_compute(
                            "ReduceScatter", OP.add, replica_groups=rgroups,
                            ins=[bounce_in.opt()], outs=[bounce_rs.opt()])

                    # ---- av on local quarter ----
                    avw = sw[f"avw{i}"]
                    avb = sw[f"avb{i}"]
                    bw = next(c for c in (500, 512, 400, 256, 250, 200, 128, 100,
                                          64, 50, 40, 25, 20, 10, 5, 4, 2, 1)
                              if blk % c == 0)
                    for bo in range(0, blk, bw):
                        cmp_ = chk.tile([H, 3, bw], bf16, tag="cmp")
                        nc.sync.dma_start(
                            out=cmp_[:],
                            in_=bounce_rs[:, :, bo:bo + bw].transpose([1, 0, 2]))
                        cml = chk.tile([H, 3, bw], bf16, tag="cml")
                        for mt in range(3):
                            pp = ps.tile([128, 4, 512], f32, tag="ps")
                            for kt in range(3):
                                nc.tensor.matmul(pp[:, 0, 0:bw], avw[:, kt, mt, :],
                                                 cmp_[:, kt, :], start=(kt == 0),
                                                 stop=(kt == 2))
                            if av_bnz:
                                nc.vector.tensor_scalar_add(
                                    pp[:, 0, 0:bw], pp[:, 0, 0:bw],
                                    avb[:, mt:mt + 1])
                            rl2 = chk.tile([H, bw], f32, tag="rl2")
                            nc.scalar.activation(
                                out=rl2[:], in_=pp[:, 0, 0:bw],
                                func=AF.Relu, scale=1.0 - av_a[i])
                            nc.vector.scalar_tensor_tensor(
                                out=cml[:, mt, :], in0=pp[:, 0, 0:bw],
                                scalar=av_a[i], in1=rl2[:],
                                op0=OP.mult, op1=OP.add)
                        nc.sync.dma_start(
                            out=bounce_rs[:, :, bo:bo + bw].transpose([1, 0, 2]),
                            in_=cml[:])

                    if with_cc:
                        nc.gpsimd.collective_compute(
                            "AllGather", OP.bypass, replica_groups=rgroups,
                            ins=[bounce_rs.opt()], outs=[bounce_cm.opt()])

                    # ---- cc + chn stats ----
                    ccw = sw[f"ccw{i}"]
                    ccb = sw[f"ccb{i}"]
                    z2 = new_z()
                    stats2 = tmp.tile([N, nch, 6], f32, tag="gnstats")
                    for ci in range(nch):
                        off = ci * cw
                        chc = chk.tile([H, 3, cw], bf16, tag="chc2")
                        nc.sync.dma_start(out=chc[:], in_=ch_dram[:, :, off:off + cw])
                        cmc = chk.tile([H, 3, cw], bf16, tag="cmc")
                        dma_split_blk(bounce_cm, cmc[:], off, cw, write=False)
                        pp = ps.tile([128, 4, 512], f32, tag="ps")
                        for kt in range(3):
                            nc.tensor.matmul(pp[0:N, 0, 0:cw], ccw[:, kt, :],
                                             chc[:, kt, :], start=(kt == 0),
                                             stop=False)
                        for kt in range(3):
                            nc.tensor.matmul(pp[0:N, 0, 0:cw], ccw[:, 3 + kt, :],
                                             cmc[:, kt, :], start=False,
                                             stop=(kt == 2))
                        if cc_bnz:
                            nc.vector.tensor_scalar_add(
                                pp[0:N, 0, 0:cw], pp[0:N, 0, 0:cw], ccb[:])
                        rl3 = chk.tile([N, cw], f32, tag="rl3")
                        nc.scalar.activation(out=rl3[:], in_=pp[0:N, 0, 0:cw],
                                             func=AF.Relu, scale=1.0 - cc_a[i])
                        nc.vector.scalar_tensor_tensor(
                            out=z2[:, off:off + cw], in0=pp[0:N, 0, 0:cw],
                            scalar=cc_a[i], in1=rl3[:], op0=OP.mult, op1=OP.add)
                        nc.vector.bn_stats(out=stats2[:, ci, :],
                                           in_=z2[:, off:off + cw])
                    s2, t2 = gn_finalize(stats2, sw[f"chnw{i}"], sw[f"chnb{i}"])

                    if i < layers - 1:
                        x65 = new_x65()
                        if has_xbias:
                            nc.vector.memset(x65[N:KX, :], 1.0)
                        apply_chunks(z2, s2, t2, cast_consumer(x65), cw, nch)
                    else:
                        outw = sw["outw"]
                        outb = sw["outb"]

                        def fin_consumer(ci, off, outc):
                            rl4 = chk.tile([N, cw], f32, tag="rl4")
                            nc.scalar.activation(out=rl4[:], in_=outc[:],
                                                 func=AF.Relu, scale=1.0 - out_a)
                            yp = chk.tile([N, cw], f32r, tag="yp")
                            nc.vector.scalar_tensor_tensor(
                                out=yp[:], in0=outc[:], scalar=out_a,
                                in1=rl4[:], op0=OP.mult, op1=OP.add)
                            pp = ps.tile([128, 4, 512], f32, tag="ps")
                            nc.tensor.matmul(pp[0:OUT, 0, 0:cw], outw[:], yp[:],
                                             start=True, stop=True)
                            yc = chk.tile([OUT, cw], f32, tag="yc")
                            nc.vector.tensor_scalar_add(yc[:], pp[0:OUT, 0, 0:cw],
                                                        outb[:])
                            nc.sync.dma_start(out=y_out[:, off:off + cw], in_=yc[:])

                        apply_chunks(z2, s2, t2, fin_consumer, cw, nch)


            if n_iter == 1:
                _network()
            else:
                with tc.For_i(0, n_iter, 1):
                    _network()

    nc.compile()
    return nc


def make_in_maps(inputs, d1=D1, d2=D2, layers=L):
    x = np.asarray(inputs["x"], np.float32)
    per_core = []
    has_xbias = None
    for b in range(B):
        for c in range(CH):
            w = prep_weights(inputs, b, c, layers)
            has_xbias = w.pop("_has_xbias")
            m = {"x": np.ascontiguousarray(x[b, c].reshape(N, d1 * d2))}
            for k, v in w.items():
                m[k] = np.ascontiguousarray(v)
            per_core.append(m)
    alphas = (
        [float(np.asarray(inputs["tr_a"][i])) for i in range(layers)],
        [float(np.asarray(inputs["av_a"][i])) for i in range(layers)],
        [float(np.asarray(inputs["cc_a"][i])) for i in range(layers)],
        float(np.asarray(inputs["out_a"])),
    )
    bias_flags = tuple(
        bool(np.any(np.asarray(inputs[k]) != 0))
        for k in ("tr_b", "av_b", "cc_b"))
    return per_core, has_xbias, alphas, bias_flags


def kernel(**inputs):
    import concourse.bass_utils as bass_utils

    per_core, has_xbias, alphas, bias_flags = make_in_maps(inputs)
    ck = (has_xbias, tuple(map(tuple, alphas[:3])), alphas[3], bias_flags)
    if ck not in _CACHE:
        _CACHE[ck] = build_program(has_xbias, alphas, bias_flags)
    nc = _CACHE[ck]
    r = bass_utils.run_bass_kernel_spmd(nc, per_core, core_ids=list(range(NCORES)))
    ys = np.zeros((B * CH, OUT, D1, D2), np.float32)
    for ci in range(NCORES):
        ys[ci] = r.results[ci]["y"].reshape(OUT, D1, D2)
    return ys



# revision 9
# speedup vs baseline: 1.5888x; 1.5888x over previous
"""Trainium2 Bass kernel for nn_DPRNN_TAC (DPRNN + TAC, L=2 layers).

Sharding: one (batch, channel) pair per NeuronCore (B*CH = 8 = n_cores).
Row/col BiLSTMs, group norms and TAC MLPs are core-local; the TAC channel
mean is a ReduceScatter + AllGather over the 4 cores of each batch group.

Math tricks (validated against the fp32 reference in a numpy golden model):
 - All LSTM gate nonlinearities are evaluated with tanh only
   (sigmoid(x) = 0.5 + 0.5*tanh(x/2)); the 0.5 input scale for gates
   i, f, o is folded into the weights host-side.
 - Stored hidden state is 2h; the 0.5 correction is folded into Whh and
   the projection weights host-side.
 - The masked channel mean divides by eff = num_mic[b]; 1/eff is folded
   into av_w host-side, and cores with channel >= eff contribute zeros.
Precision: scan weights/activations bf16 (fp32 psum + fp32 cell state),
big MLPs float32r, group-norm statistics fp32.

Memory: the residual stream ("out") and the BiLSTM hidden histories live
in HBM and are streamed in chunks; SBUF holds the scan working set, the
current x-cast (bf16) and the pre-norm z tensor.
"""

import numpy as np
import ml_dtypes

BF16 = ml_dtypes.bfloat16

L, B, CH, N, H, D1, D2, OUT = 2, 2, 4, 64, 128, 100, 200, 64
E = 3 * H
NCORES = 8

_CACHE = {}


def prep_weights(inp, core_b, core_c, layers=L):
    """Host-side weight transforms for one core. Returns dict[str, np.ndarray]."""
    gs = np.array([0.5, 0.5, 1.0, 0.5], np.float32)  # gate scales i,f,g,o
    num_mic = np.asarray(inp["num_mic"]).astype(np.int64)
    eff = int(num_mic[core_b]) if int(num_mic.max()) > 0 else CH
    if eff <= 0:
        eff = CH
    w = {}
    scan_bias_nonzero = False
    for r in ("row", "col"):
        for i in range(layers):
            bsum = np.asarray(inp[f"{r}_bih"][i]) + np.asarray(inp[f"{r}_bhh"][i])
            if np.any(np.asarray(bsum) != 0):
                scan_bias_nonzero = True
    w["_has_xbias"] = scan_bias_nonzero
    KX = 65 if scan_bias_nonzero else 64

    for r in ("row", "col"):
        for i in range(layers):
            whh = np.zeros((2, 4, H, H), np.float32)
            wxb = np.zeros((2, 4, KX, H), np.float32)
            for d in range(2):
                Wih = np.asarray(inp[f"{r}_Wih"][i][d], np.float32).reshape(4, H, N)
                Whh = np.asarray(inp[f"{r}_Whh"][i][d], np.float32).reshape(4, H, H)
                bsum = (
                    np.asarray(inp[f"{r}_bih"][i][d], np.float32)
                    + np.asarray(inp[f"{r}_bhh"][i][d], np.float32)
                ).reshape(4, H)
                for g in range(4):
                    whh[d, g] = (Whh[g] * gs[g] * 0.5).T  # lhsT [h, gh]
                    wx = (Wih[g] * gs[g]).T  # [N, H]
                    if scan_bias_nonzero:
                        wxb[d, g] = np.vstack([wx, (bsum[g] * gs[g])[None, :]])
                    else:
                        wxb[d, g] = wx
            # SBUF layout: partition dim first
            w[f"whh_{r}{i}"] = np.moveaxis(whh, 2, 0).astype(BF16)  # [H,2,4,H]
            w[f"wxb_{r}{i}"] = np.moveaxis(wxb, 2, 0).astype(BF16)  # [KX,2,4,H]
            pw = np.asarray(inp[f"{r}_proj_w"][i], np.float32)  # [N, 2H]
            pj = np.zeros((2, H, N), np.float32)
            pj[0] = (0.5 * pw[:, :H]).T
            pj[1] = (0.5 * pw[:, H:]).T
            w[f"proj_{r}{i}"] = np.moveaxis(pj, 1, 0).astype(BF16)  # [H,2,N]
            w[f"pb_{r}{i}"] = np.asarray(
                inp[f"{r}_proj_b"][i], np.float32).reshape(N, 1)
    for i in range(layers):
        trw = np.asarray(inp["tr_w"][i], np.float32)  # [E, N]
        w[f"trw{i}"] = np.ascontiguousarray(trw.T.reshape(N, 3, H))
        w[f"trb{i}"] = np.ascontiguousarray(
            np.asarray(inp["tr_b"][i], np.float32).reshape(3, H).T)  # [H,3]
        avw = np.asarray(inp["av_w"][i], np.float32) / float(eff)  # [E, E]
        avw4 = np.ascontiguousarray(avw.T.reshape(3, H, 3, H))  # [kt,k,mt,m]
        w[f"avw{i}"] = np.moveaxis(avw4, 1, 0).astype(BF16)  # [H,kt,mt,m]
        w[f"avb{i}"] = np.ascontiguousarray(
            np.asarray(inp["av_b"][i], np.float32).reshape(3, H).T)  # [H,3]
        ccw = np.asarray(inp["cc_w"][i], np.float32)  # [N, 2E]
        ccw6 = np.ascontiguousarray(ccw.T.reshape(6, H, N))
        w[f"ccw{i}"] = np.moveaxis(ccw6, 1, 0).astype(BF16)  # [H,6,N]
        w[f"ccb{i}"] = np.asarray(inp["cc_b"][i], np.float32).reshape(N, 1)
        for nm in ("rn", "cn", "chn"):
            w[f"{nm}w{i}"] = np.asarray(inp[f"{nm}_w"][i], np.float32).reshape(N, 1)
            w[f"{nm}b{i}"] = np.asarray(inp[f"{nm}_b"][i], np.float32).reshape(N, 1)
    w["outw"] = np.ascontiguousarray(np.asarray(inp["out_w"], np.float32).T)
    w["outb"] = np.asarray(inp["out_b"], np.float32).reshape(OUT, 1)
    w["msk"] = np.full((H, 1), 1.0 if core_c < eff else 0.0, np.float32)
    return w


def build_program(has_xbias, alphas, bias_flags, d1=D1, d2=D2, layers=L,
                  n_cores=NCORES, n_iter=1, with_cc=True, n_streams=2):
    import concourse.bass as bass
    import concourse.tile as tile
    from concourse import bacc, mybir
    import contextlib

    f32 = mybir.dt.float32
    f32r = mybir.dt.float32r
    bf16 = mybir.dt.bfloat16
    AF = mybir.ActivationFunctionType
    OP = mybir.AluOpType

    pos = d1 * d2
    assert pos % 4 == 0
    blk = pos // 4  # allreduce block per group rank
    # chunk width for p-major pointwise loops
    cw = next(c for c in (512, 500, 400, 256, 200, 128, 100, 64, 48, 32, 20, 12, 8, 4)
              if pos % c == 0)
    nch = pos // cw
    # cn-apply chunk: whole p-rows, <=512
    rows_per = max(1, 400 // d2) if d2 <= 400 else 1
    cwc = rows_per * d2
    while pos % cwc != 0:
        rows_per -= 1
        cwc = rows_per * d2
    nchc = pos // cwc
    tr_a, av_a, cc_a, out_a = alphas
    tr_bnz, av_bnz, cc_bnz = bias_flags
    KX = 65 if has_xbias else 64
    n_groups = n_cores // 4
    rgroups = [[g * 4 + j for j in range(4)] for g in range(n_groups)]

    nc = bacc.Bacc("TRN2", target_bir_lowering=False, debug=False,
                   num_devices=n_cores)

    x_in = nc.dram_tensor("x", [N, pos], f32, kind="ExternalInput")
    y_out = nc.dram_tensor("y", [OUT, pos], f32, kind="ExternalOutput")

    def din(name, shape, dt):
        return nc.dram_tensor(name, shape, dt, kind="ExternalInput")

    wt = {}
    for r in ("row", "col"):
        for i in range(layers):
            wt[f"whh_{r}{i}"] = din(f"whh_{r}{i}", [H, 2, 4, H], bf16)
            wt[f"wxb_{r}{i}"] = din(f"wxb_{r}{i}", [KX, 2, 4, H], bf16)
            wt[f"proj_{r}{i}"] = din(f"proj_{r}{i}", [H, 2, N], bf16)
            wt[f"pb_{r}{i}"] = din(f"pb_{r}{i}", [N, 1], f32)
    for i in range(layers):
        wt[f"trw{i}"] = din(f"trw{i}", [N, 3, H], f32r)
        wt[f"trb{i}"] = din(f"trb{i}", [H, 3], f32)
        wt[f"avw{i}"] = din(f"avw{i}", [H, 3, 3, H], bf16)
        wt[f"avb{i}"] = din(f"avb{i}", [H, 3], f32)
        wt[f"ccw{i}"] = din(f"ccw{i}", [H, 6, N], bf16)
        wt[f"ccb{i}"] = din(f"ccb{i}", [N, 1], f32)
        for nm in ("rn", "cn", "chn"):
            wt[f"{nm}w{i}"] = din(f"{nm}w{i}", [N, 1], f32)
            wt[f"{nm}b{i}"] = din(f"{nm}b{i}", [N, 1], f32)
    wt["outw"] = din("outw", [N, OUT], f32r)
    wt["outb"] = din("outb", [OUT, 1], f32)
    wt["msk"] = din("msk", [H, 1], f32)

    with tile.TileContext(nc) as tc:
        with contextlib.ExitStack() as ctx:
            singles = ctx.enter_context(tc.tile_pool(name="singles", bufs=1))
            xz = ctx.enter_context(tc.tile_pool(name="xz", bufs=2))
            ps = ctx.enter_context(tc.tile_pool(name="ps", bufs=2, space="PSUM"))
            pscan = ctx.enter_context(tc.tile_pool(name="pscan", bufs=1, space="PSUM"))
            tmp = ctx.enter_context(tc.tile_pool(name="tmp", bufs=2))
            chk = ctx.enter_context(tc.tile_pool(name="chk", bufs=2))
            dram = ctx.enter_context(tc.tile_pool(name="dram", bufs=1, space="DRAM"))

            sw = {}
            for k, t in wt.items():
                sw[k] = singles.tile(list(t.shape), t.dtype, tag=f"w_{k}", name=f"sw_{k}")
                nc.sync.dma_start(out=sw[k][:], in_=t[:])

            out_dram = dram.tile([N, pos], f32r)
            ch_dram = dram.tile([H, 3, pos], bf16)
            hf_dram = dram.tile([H, pos], bf16)
            hb_dram = dram.tile([H, pos], bf16)
            bounce_in = dram.tile([4, 3, H, blk], bf16)
            bounce_rs = dram.tile([3, H, blk], bf16)
            bounce_cm = dram.tile([4, 3, H, blk], bf16)
            gn_dram = dram.tile([1, 2], f32)

            def new_x65():
                return xz.tile([KX, pos], bf16, tag="xz", name="x65t")

            def new_z():
                return xz.tile([N, pos], bf16, tag="xz", name="zt")

            ones_k = singles.tile([N, 1], f32, tag="ones_k")
            nc.vector.memset(ones_k[:], 1.0)
            epsc = singles.tile([1, 1], f32, tag="epsc")
            nc.vector.memset(epsc[:], 1e-8)

            def gn_finalize(stats, wv, bv):
                """stats [N, nchunks, 6] -> per-partition s,t [N,1] f32."""
                mv = tmp.tile([N, 2], f32, tag="gnmv")
                nc.vector.bn_aggr(out=mv[:], in_=stats[:])
                r3 = tmp.tile([N, 3], f32, tag="gnr3")
                nc.vector.tensor_copy(r3[:, 0:2], mv[:, 0:2])
                nc.vector.tensor_mul(r3[:, 2:3], mv[:, 0:1], mv[:, 0:1])
                pr = ps.tile([128, 512], f32, tag="ps")
                nc.tensor.matmul(pr[0:1, 0:3], ones_k[:], r3[:],
                                 start=True, stop=True)
                sc = tmp.tile([1, 3], f32, tag="gnsc")
                nc.vector.tensor_scalar_mul(sc[0:1, 0:3], pr[0:1, 0:3], 1.0 / N)
                var = tmp.tile([1, 1], f32, tag="gnvar")
                nc.vector.tensor_mul(var[0:1], sc[0:1, 0:1], sc[0:1, 0:1])
                nc.vector.tensor_sub(var[0:1], sc[0:1, 1:2], var[0:1])
                nc.vector.tensor_add(var[0:1], var[0:1], sc[0:1, 2:3])
                # wait: var = S1/N - mu^2 + S2/N  (S1=sum var_p, S2=sum mu_p^2)
                rm = tmp.tile([1, 2], f32, tag="gnrm")
                nc.scalar.activation(out=rm[0:1, 0:1], in_=var[0:1], func=AF.Sqrt,
                                     bias=epsc[0:1, :], scale=1.0)
                nc.vector.reciprocal(rm[0:1, 0:1], rm[0:1, 0:1])
                nc.vector.tensor_scalar_mul(rm[0:1, 1:2], sc[0:1, 0:1], -1.0)
                nc.sync.dma_start(out=gn_dram[0:1, :], in_=rm[0:1, :])
                rb = tmp.tile([N, 2], f32, tag="gnrb")
                bcast = bass.AP(tensor=gn_dram[:].tensor, offset=gn_dram[:].offset,
                                ap=[[0, N], [1, 2]])
                nc.sync.dma_start(out=rb[:], in_=bcast)
                s = tmp.tile([N, 1], f32, tag="gns")
                t = tmp.tile([N, 1], f32, tag="gnt")
                nc.vector.tensor_mul(s[:], wv[:], rb[:, 0:1])
                nc.vector.scalar_tensor_tensor(
                    out=t[:], in0=s[:], scalar=rb[:, 1:2], in1=bv[:],
                    op0=OP.mult, op1=OP.add)
                return s, t

            # ---------------- LSTM scan ----------------
            def scan(r, i, x65t, T, M, x_ap, hdst):
                """Software-pipelined scan over phase-shifted streams
                (direction x column-range). Round-robin emission lets the
                scheduler overlap one stream's ACT/DVE chain with another
                stream's matmuls. x_ap(d, k, c0, Ms) -> AP [KX, Ms];
                hdst(d, seq, c0, Ms) -> DRAM AP [H, Ms]."""
                whh = sw[f"whh_{r}{i}"]
                wxb = sw[f"wxb_{r}{i}"]
                nh = max(1, n_streams // 2)
                csz = M // nh
                assert csz * nh == M
                streams = [(d, h * csz) for d in range(2) for h in range(nh)]
                GS = 128 if csz <= 128 else 256
                st = []
                for si, (d, c0) in enumerate(streams):
                    wst = tmp.tile([H, csz], f32, tag=f"wst{si}",
                                   name=f"wst{si}")
                    nc.vector.memset(wst[:], 0.0)
                    hr = tmp.tile([H, 2, csz], bf16, tag=f"hr{si}",
                                  name=f"hr{si}")
                    st.append((wst, hr))
                for k in range(T):
                    for si, (d, c0) in enumerate(streams):
                        wst, hr = st[si]
                        seq = k if d == 0 else (T - 1 - k)
                        pst = pscan.tile([128, 4, GS], f32, tag=f"pss{si}",
                                         name=f"pss{si}")
                        for g in range(4):
                            # start=True clears has_written for the WHOLE
                            # bank; only the first matmul into each bank may
                            # set it (later gates in the bank find cleared
                            # bits and overwrite, then hh accumulates).
                            first_in_bank = (
                                g == 0 or (g * GS) // 512 != ((g - 1) * GS) // 512)
                            nc.tensor.matmul(
                                pst[:, g, 0:csz], wxb[:, d, g, :],
                                x_ap(d, k, c0, csz),
                                start=first_in_bank, stop=(k == 0))
                        if k > 0:
                            for g in range(4):
                                nc.tensor.matmul(
                                    pst[:, g, 0:csz], whh[:, d, g, :],
                                    hr[:, (k - 1) % 2, :],
                                    start=False, stop=True)
                        Tt = tmp.tile([H, 4, csz], f32, tag=f"Tt{si}",
                                      name=f"Tt{si}")
                        nc.scalar.activation(out=Tt[:], in_=pst[:, :, 0:csz],
                                             func=AF.Tanh)
                        pt = tmp.tile([H, csz], f32, tag=f"pt{si}",
                                      name=f"pt{si}")
                        qt = tmp.tile([H, csz], f32, tag=f"qt{si}",
                                      name=f"qt{si}")
                        nc.vector.scalar_tensor_tensor(
                            out=qt[:], in0=Tt[:, 0, :], scalar=1.0,
                            in1=Tt[:, 2, :], op0=OP.add, op1=OP.mult)
                        nc.vector.scalar_tensor_tensor(
                            out=pt[:], in0=Tt[:, 1, :], scalar=1.0,
                            in1=wst[:], op0=OP.add, op1=OP.mult)
                        nc.vector.scalar_tensor_tensor(
                            out=wst[:], in0=pt[:], scalar=0.5,
                            in1=qt[:], op0=OP.mult, op1=OP.add)
                        Tc = tmp.tile([H, csz], f32, tag=f"Tc{si}",
                                      name=f"Tc{si}")
                        nc.scalar.activation(out=Tc[:], in_=wst[:],
                                             func=AF.Tanh, scale=0.5)
                        nc.vector.scalar_tensor_tensor(
                            out=hr[:, k % 2, :], in0=Tt[:, 3, :],
                            scalar=1.0, in1=Tc[:], op0=OP.add, op1=OP.mult)
                        nc.sync.dma_start(out=hdst(d, seq, c0, csz),
                                          in_=hr[:, k % 2, :])

            # ------------- proj + GN stats -------------
            def proj_phase(r, i, nchunks, cwp, gnw, gnb):
                """Ĥ streamed from HBM in contiguous cwp chunks."""
                z = new_z()
                pj = sw[f"proj_{r}{i}"]
                pb = sw[f"pb_{r}{i}"]
                stats = tmp.tile([N, nchunks, 6], f32, tag="gnstats")
                for ci in range(nchunks):
                    off = ci * cwp
                    hfc = chk.tile([H, cwp], bf16, tag="hfc")
                    hbc = chk.tile([H, cwp], bf16, tag="hbc")
                    nc.sync.dma_start(out=hfc[:], in_=hf_dram[:, off:off + cwp])
                    nc.sync.dma_start(out=hbc[:], in_=hb_dram[:, off:off + cwp])
                    pp = ps.tile([128, 512], f32, tag="ps")
                    nc.tensor.matmul(pp[0:N, 0:cwp], pj[:, 0, :], hfc[:],
                                     start=True, stop=False)
                    nc.tensor.matmul(pp[0:N, 0:cwp], pj[:, 1, :], hbc[:],
                                     start=False, stop=True)
                    nc.vector.tensor_scalar_add(z[:, off:off + cwp],
                                                pp[0:N, 0:cwp], pb[:])
                    nc.vector.bn_stats(out=stats[:, ci, :], in_=z[:, off:off + cwp])
                s, t = gn_finalize(stats, gnw, gnb)
                return z, s, t

            # ---- chunked apply: out' = out + z*s + t, fused consumer ----
            def apply_chunks(z, s, t, consumer, cwa, ncha, z_ap=None,
                             first_src=None):
                for ci in range(ncha):
                    off = ci * cwa
                    outc = chk.tile([N, cwa], f32r, tag="outc")
                    src = first_src if first_src is not None else out_dram
                    nc.sync.dma_start(out=outc[:],
                                      in_=src[:, off:off + cwa].bitcast(f32r))
                    if z is not None:
                        v = chk.tile([N, cwa], bf16, tag="vt")
                        zin = z_ap(z, ci) if z_ap is not None else z[:, off:off + cwa]
                        nc.vector.tensor_scalar(
                            out=v[:], in0=zin, scalar1=s[:], scalar2=t[:],
                            op0=OP.mult, op1=OP.add)
                        nc.vector.tensor_add(outc[:], outc[:], v[:])
                        nc.sync.dma_start(out=out_dram[:, off:off + cwa],
                                          in_=outc[:])
                    consumer(ci, off, outc)

            def cast_consumer(x65t):
                def f(ci, off, outc):
                    nc.gpsimd.tensor_copy(out=x65t[0:N, off:off + outc.shape[1]],
                                          in_=outc[:])
                return f

            def dma_split_blk(dst4, sb, off, width, write=True):
                """DMA [H,3,width] SBUF <-> bounce[4,3,H,blk], splitting at
                block boundaries."""
                done = 0
                while done < width:
                    o = off + done
                    q, qo = o // blk, o % blk
                    wseg = min(width - done, blk - qo)
                    dr = dst4[q, :, :, qo:qo + wseg].transpose([1, 0, 2])
                    if write:
                        nc.sync.dma_start(out=dr, in_=sb[:, :, done:done + wseg])
                    else:
                        nc.sync.dma_start(out=sb[:, :, done:done + wseg], in_=dr)
                    done += wseg

            # ================= network =================
            def _network():
                x65 = new_x65()
                if has_xbias:
                    nc.vector.memset(x65[N:KX, :], 1.0)

                def init_consumer(x65t):
                    def f(ci, off, outc):
                        nc.sync.dma_start(out=out_dram[:, off:off + cw], in_=outc[:])
                        nc.gpsimd.tensor_copy(out=x65t[0:N, off:off + cw], in_=outc[:])
                    return f

                apply_chunks(None, None, None, init_consumer(x65), cw, nch,
                             first_src=x_in)

                for i in range(layers):
                    # ---- row: sequences along d1 (t=p), batch d2 ----
                    scan("row", i, x65, d1, d2,
                         lambda d, k, c0, Ms: x65[
                             0:KX,
                             (k if d == 0 else d1 - 1 - k) * d2 + c0:
                             (k if d == 0 else d1 - 1 - k) * d2 + c0 + Ms],
                         lambda d, seq, c0, Ms: (hf_dram if d == 0 else hb_dram)[
                             :, seq * d2 + c0:seq * d2 + c0 + Ms])
                    z, s, t = proj_phase("row", i, nch, cw, sw[f"rnw{i}"], sw[f"rnb{i}"])
                    x65c = new_x65()
                    if has_xbias:
                        nc.vector.memset(x65c[N:KX, :], 1.0)
                    apply_chunks(z, s, t, cast_consumer(x65c), cw, nch)

                    # ---- col: sequences along d2 (t=q), batch d1; strided x ----
                    def xcol(d, k, c0, Ms, _x=x65c):
                        q = k if d == 0 else d2 - 1 - k
                        a = _x[0:KX, q + c0 * d2:q + c0 * d2 + 1]
                        return bass.AP(tensor=a.tensor, offset=a.offset,
                                       ap=[a.ap[0], [d2, Ms]])

                    scan("col", i, x65c, d2, d1, xcol,
                         lambda d, seq, c0, Ms: (hf_dram if d == 0 else hb_dram)[
                             :, seq * d1 + c0:seq * d1 + c0 + Ms])
                    # col proj in q-major contiguous chunks; z stored q-major
                    zq, s2_, t2_ = proj_phase("col", i, nch, cw,
                                              sw[f"cnw{i}"], sw[f"cnb{i}"])

                    # cn-apply in p-row-aligned chunks with strided z view,
                    # fused with TAC tr + mask + bounce write
                    def zq_ap(zt, ci, _rp=rows_per):
                        p0 = ci * _rp
                        a = zt[:, p0:p0 + 1]
                        return bass.AP(tensor=a.tensor, offset=a.offset,
                                       ap=[a.ap[0], [1, _rp], [d1, d2]])

                    trw = sw[f"trw{i}"]
                    trb = sw[f"trb{i}"]
                    msk = sw["msk"]

                    def tr_consumer(ci, off, outc, _i=i):
                        wdt = outc.shape[1]
                        chc = chk.tile([H, 3, cwc], bf16, tag="chc")
                        for e in range(3):
                            pp = ps.tile([128, 512], f32, tag="ps")
                            nc.tensor.matmul(
                                pp[:, 0:wdt], trw[:, e, :],
                                outc[:], start=True, stop=True)
                            if tr_bnz:
                                nc.vector.tensor_scalar_add(
                                    pp[:, 0:wdt], pp[:, 0:wdt],
                                    trb[:, e:e + 1])
                            rl = chk.tile([H, cwc], f32, tag="rl")
                            nc.scalar.activation(
                                out=rl[:, 0:wdt], in_=pp[:, 0:wdt],
                                func=AF.Relu, scale=1.0 - tr_a[_i])
                            nc.vector.scalar_tensor_tensor(
                                out=chc[:, e, 0:wdt], in0=pp[:, 0:wdt],
                                scalar=tr_a[_i], in1=rl[:, 0:wdt],
                                op0=OP.mult, op1=OP.add)
                        nc.sync.dma_start(out=ch_dram[:, :, off:off + wdt],
                                          in_=chc[:, :, 0:wdt])
                        con = chk.tile([H, 3, cwc], bf16, tag="conc")
                        nc.vector.tensor_scalar_mul(con[:, :, 0:wdt],
                                                    chc[:, :, 0:wdt], msk[:])
                        dma_split_blk(bounce_in, con[:, :, 0:wdt], off, wdt)

                    apply_chunks(zq, s2_, t2_, tr_consumer, cwc, nchc, z_ap=zq_ap)

                    if with_cc:
                        nc.gpsimd.collective_compute(
                            "ReduceScatter", OP.add, replica_groups=rgroups,
                            ins=[bounce_in.opt()], outs=[bounce_rs.opt()])

                    # ---- av on local quarter ----
                    avw = sw[f"avw{i}"]
                    avb = sw[f"avb{i}"]
                    bw = next(c for c in (500, 512, 400, 256, 250, 200, 128, 100,
                                          64, 50, 40, 25, 20, 10, 5, 4, 2, 1)
                              if blk % c == 0)
                    for bo in range(0, blk, bw):
                        cmp_ = chk.tile([H, 3, bw], bf16, tag="cmp")
                        nc.sync.dma_start(
                            out=cmp_[:],
                            in_=bounce_rs[:, :, bo:bo + bw].transpose([1, 0, 2]))
                        cml = chk.tile([H, 3, bw], bf16, tag="cml")
                        for mt in range(3):
                            pp = ps.tile([128, 512], f32, tag="ps")
                            for kt in range(3):
                                nc.tensor.matmul(pp[:, 0:bw], avw[:, kt, mt, :],
                                                 cmp_[:, kt, :], start=(kt == 0),
                                                 stop=(kt == 2))
                            if av_bnz:
                                nc.vector.tensor_scalar_add(
                                    pp[:, 0:bw], pp[:, 0:bw],
                                    avb[:, mt:mt + 1])
                            rl2 = chk.tile([H, bw], f32, tag="rl2")
                            nc.scalar.activation(
                                out=rl2[:], in_=pp[:, 0:bw],
                                func=AF.Relu, scale=1.0 - av_a[i])
                            nc.vector.scalar_tensor_tensor(
                                out=cml[:, mt, :], in0=pp[:, 0:bw],
                                scalar=av_a[i], in1=rl2[:],
                                op0=OP.mult, op1=OP.add)
                        nc.sync.dma_start(
                            out=bounce_rs[:, :, bo:bo + bw].transpose([1, 0, 2]),
                            in_=cml[:])

                    if with_cc:
                        nc.gpsimd.collective_compute(
                            "AllGather", OP.bypass, replica_groups=rgroups,
                            ins=[bounce_rs.opt()], outs=[bounce_cm.opt()])

                    # ---- cc + chn stats ----
                    ccw = sw[f"ccw{i}"]
                    ccb = sw[f"ccb{i}"]
                    z2 = new_z()
                    stats2 = tmp.tile([N, nch, 6], f32, tag="gnstats")
                    for ci in range(nch):
                        off = ci * cw
                        chc = chk.tile([H, 3, cw], bf16, tag="chc2")
                        nc.sync.dma_start(out=chc[:], in_=ch_dram[:, :, off:off + cw])
                        cmc = chk.tile([H, 3, cw], bf16, tag="cmc")
                        dma_split_blk(bounce_cm, cmc[:], off, cw, write=False)
                        pp = ps.tile([128, 512], f32, tag="ps")
                        for kt in range(3):
                            nc.tensor.matmul(pp[0:N, 0:cw], ccw[:, kt, :],
                                             chc[:, kt, :], start=(kt == 0),
                                             stop=False)
                        for kt in range(3):
                            nc.tensor.matmul(pp[0:N, 0:cw], ccw[:, 3 + kt, :],
                                             cmc[:, kt, :], start=False,
                                             stop=(kt == 2))
                        if cc_bnz:
                            nc.vector.tensor_scalar_add(
                                pp[0:N, 0:cw], pp[0:N, 0:cw], ccb[:])
                        rl3 = chk.tile([N, cw], f32, tag="rl3")
                        nc.scalar.activation(out=rl3[:], in_=pp[0:N, 0:cw],
                                             func=AF.Relu, scale=1.0 - cc_a[i])
                        nc.vector.scalar_tensor_tensor(
                            out=z2[:, off:off + cw], in0=pp[0:N, 0:cw],
                            scalar=cc_a[i], in1=rl3[:], op0=OP.mult, op1=OP.add)
                        nc.vector.bn_stats(out=stats2[:, ci, :],
                                           in_=z2[:, off:off + cw])
                    s2, t2 = gn_finalize(stats2, sw[f"chnw{i}"], sw[f"chnb{i}"])

                    if i < layers - 1:
                        x65 = new_x65()
                        if has_xbias:
                            nc.vector.memset(x65[N:KX, :], 1.0)
                        apply_chunks(z2, s2, t2, cast_consumer(x65), cw, nch)
                    else:
                        outw = sw["outw"]
                        outb = sw["outb"]

                        def fin_consumer(ci, off, outc):
                            rl4 = chk.tile([N, cw], f32, tag="rl4")
                            nc.scalar.activation(out=rl4[:], in_=outc[:],
                                                 func=AF.Relu, scale=1.0 - out_a)
                            yp = chk.tile([N, cw], f32r, tag="yp")
                            nc.vector.scalar_tensor_tensor(
                                out=yp[:], in0=outc[:], scalar=out_a,
                                in1=rl4[:], op0=OP.mult, op1=OP.add)
                            pp = ps.tile([128, 512], f32, tag="ps")
                            nc.tensor.matmul(pp[0:OUT, 0:cw], outw[:], yp[:],
                                             start=True, stop=True)
                            yc = chk.tile([OUT, cw], f32, tag="yc")
                            nc.vector.tensor_scalar_add(yc[:], pp[0:OUT, 0:cw],
                                                        outb[:])
                            nc.sync.dma_start(out=y_out[:, off:off + cw], in_=yc[:])

                        apply_chunks(z2, s2, t2, fin_consumer, cw, nch)


            if n_iter == 1:
                _network()
            else:
                with tc.For_i(0, n_iter, 1):
                    _network()

    nc.compile()
    return nc


def make_in_maps(inputs, d1=D1, d2=D2, layers=L):
    x = np.asarray(inputs["x"], np.float32)
    per_core = []
    has_xbias = None
    for b in range(B):
        for c in range(CH):
            w = prep_weights(inputs, b, c, layers)
            has_xbias = w.pop("_has_xbias")
            m = {"x": np.ascontiguousarray(x[b, c].reshape(N, d1 * d2))}
            for k, v in w.items():
                m[k] = np.ascontiguousarray(v)
            per_core.append(m)
    alphas = (
        [float(np.asarray(inputs["tr_a"][i])) for i in range(layers)],
        [float(np.asarray(inputs["av_a"][i])) for i in range(layers)],
        [float(np.asarray(inputs["cc_a"][i])) for i in range(layers)],
        float(np.asarray(inputs["out_a"])),
    )
    bias_flags = tuple(
        bool(np.any(np.asarray(inputs[k]) != 0))
        for k in ("tr_b", "av_b", "cc_b"))
    return per_core, has_xbias, alphas, bias_flags


def kernel(**inputs):
    import os
    import concourse.bass_utils as bass_utils

    n_streams = int(os.environ.get("BASS_NSTREAM", "2"))
    per_core, has_xbias, alphas, bias_flags = make_in_maps(inputs)
    ck = (has_xbias, tuple(map(tuple, alphas[:3])), alphas[3], bias_flags,
          n_streams)
    if ck not in _CACHE:
        _CACHE[ck] = build_program(has_xbias, alphas, bias_flags,
                                   n_streams=n_streams)
    nc = _CACHE[ck]
    r = bass_utils.run_bass_kernel_spmd(nc, per_core, core_ids=list(range(NCORES)))
    ys = np.zeros((B * CH, OUT, D1, D2), np.float32)
    for ci in range(NCORES):
        ys[ci] = r.results[ci]["y"].reshape(OUT, D1, D2)
    return ys



# revision 17
# speedup vs baseline: 1.6502x; 1.0386x over previous
"""Trainium2 Bass kernel for nn_DPRNN_TAC (DPRNN + TAC, L=2 layers).

Sharding: one (batch, channel) pair per NeuronCore (B*CH = 8 = n_cores).
Row/col BiLSTMs, group norms and TAC MLPs are core-local; the TAC channel
mean is a ReduceScatter + AllGather over the 4 cores of each batch group.

Math tricks (validated against the fp32 reference in a numpy golden model):
 - All LSTM gate nonlinearities are evaluated with tanh only
   (sigmoid(x) = 0.5 + 0.5*tanh(x/2)); the 0.5 input scale for gates
   i, f, o is folded into the weights host-side.
 - Stored hidden state is 2h; the 0.5 correction is folded into Whh and
   the projection weights host-side.
 - The masked channel mean divides by eff = num_mic[b]; 1/eff is folded
   into av_w host-side, and cores with channel >= eff contribute zeros.
Precision: scan weights/activations bf16 (fp32 psum + fp32 cell state),
big MLPs float32r, group-norm statistics fp32.

Memory: the residual stream ("out") and the BiLSTM hidden histories live
in HBM and are streamed in chunks; SBUF holds the scan working set, the
current x-cast (bf16) and the pre-norm z tensor.
"""

import numpy as np
import ml_dtypes

BF16 = ml_dtypes.bfloat16

L, B, CH, N, H, D1, D2, OUT = 2, 2, 4, 64, 128, 100, 200, 64
E = 3 * H
NCORES = 8

_CACHE = {}


def prep_weights(inp, core_b, core_c, layers=L):
    """Host-side weight transforms for one core. Returns dict[str, np.ndarray]."""
    gs = np.array([0.5, 0.5, 1.0, 0.5], np.float32)  # gate scales i,f,g,o
    num_mic = np.asarray(inp["num_mic"]).astype(np.int64)
    eff = int(num_mic[core_b]) if int(num_mic.max()) > 0 else CH
    if eff <= 0:
        eff = CH
    w = {}
    scan_bias_nonzero = False
    for r in ("row", "col"):
        for i in range(layers):
            bsum = np.asarray(inp[f"{r}_bih"][i]) + np.asarray(inp[f"{r}_bhh"][i])
            if np.any(np.asarray(bsum) != 0):
                scan_bias_nonzero = True
    w["_has_xbias"] = scan_bias_nonzero
    KX = 65 if scan_bias_nonzero else 64

    for r in ("row", "col"):
        for i in range(layers):
            whh = np.zeros((2, 4, H, H), np.float32)
            wxb = np.zeros((2, 4, KX, H), np.float32)
            for d in range(2):
                Wih = np.asarray(inp[f"{r}_Wih"][i][d], np.float32).reshape(4, H, N)
                Whh = np.asarray(inp[f"{r}_Whh"][i][d], np.float32).reshape(4, H, H)
                bsum = (
                    np.asarray(inp[f"{r}_bih"][i][d], np.float32)
                    + np.asarray(inp[f"{r}_bhh"][i][d], np.float32)
                ).reshape(4, H)
                for g in range(4):
                    whh[d, g] = (Whh[g] * gs[g] * 0.5).T  # lhsT [h, gh]
                    wx = (Wih[g] * gs[g]).T  # [N, H]
                    if scan_bias_nonzero:
                        wxb[d, g] = np.vstack([wx, (bsum[g] * gs[g])[None, :]])
                    else:
                        wxb[d, g] = wx
            # SBUF layout: partition dim first
            w[f"whh_{r}{i}"] = np.moveaxis(whh, 2, 0).astype(BF16)  # [H,2,4,H]
            w[f"wxb_{r}{i}"] = np.moveaxis(wxb, 2, 0).astype(BF16)  # [KX,2,4,H]
            pw = np.asarray(inp[f"{r}_proj_w"][i], np.float32)  # [N, 2H]
            pj = np.zeros((2, H, N), np.float32)
            pj[0] = (0.5 * pw[:, :H]).T
            pj[1] = (0.5 * pw[:, H:]).T
            w[f"proj_{r}{i}"] = np.moveaxis(pj, 1, 0).astype(BF16)  # [H,2,N]
            w[f"pb_{r}{i}"] = np.asarray(
                inp[f"{r}_proj_b"][i], np.float32).reshape(N, 1)
    for i in range(layers):
        trw = np.asarray(inp["tr_w"][i], np.float32)  # [E, N]
        w[f"trw{i}"] = np.ascontiguousarray(trw.T.reshape(N, 3, H))
        w[f"trb{i}"] = np.ascontiguousarray(
            np.asarray(inp["tr_b"][i], np.float32).reshape(3, H).T)  # [H,3]
        avw = np.asarray(inp["av_w"][i], np.float32) / float(eff)  # [E, E]
        avw4 = np.ascontiguousarray(avw.T.reshape(3, H, 3, H))  # [kt,k,mt,m]
        w[f"avw{i}"] = np.moveaxis(avw4, 1, 0).astype(BF16)  # [H,kt,mt,m]
        w[f"avb{i}"] = np.ascontiguousarray(
            np.asarray(inp["av_b"][i], np.float32).reshape(3, H).T)  # [H,3]
        ccw = np.asarray(inp["cc_w"][i], np.float32)  # [N, 2E]
        ccw6 = np.ascontiguousarray(ccw.T.reshape(6, H, N))
        w[f"ccw{i}"] = np.moveaxis(ccw6, 1, 0).astype(BF16)  # [H,6,N]
        w[f"ccb{i}"] = np.asarray(inp["cc_b"][i], np.float32).reshape(N, 1)
        for nm in ("rn", "cn", "chn"):
            w[f"{nm}w{i}"] = np.asarray(inp[f"{nm}_w"][i], np.float32).reshape(N, 1)
            w[f"{nm}b{i}"] = np.asarray(inp[f"{nm}_b"][i], np.float32).reshape(N, 1)
    w["outw"] = np.ascontiguousarray(np.asarray(inp["out_w"], np.float32).T)
    w["outb"] = np.asarray(inp["out_b"], np.float32).reshape(OUT, 1)
    w["msk"] = np.full((H, 1), 1.0 if core_c < eff else 0.0, np.float32)
    return w


def build_program(has_xbias, alphas, bias_flags, d1=D1, d2=D2, layers=L,
                  n_cores=NCORES, n_iter=1, with_cc=True, n_streams=2,
                  n_sub=5):
    import concourse.bass as bass
    import concourse.tile as tile
    from concourse import bacc, mybir
    import contextlib

    f32 = mybir.dt.float32
    f32r = mybir.dt.float32r
    bf16 = mybir.dt.bfloat16
    AF = mybir.ActivationFunctionType
    OP = mybir.AluOpType

    pos = d1 * d2
    assert pos % 4 == 0
    blk = pos // 4  # allreduce block per group rank
    sbw = blk // n_sub  # sub-block width for pipelined collectives
    assert sbw % d2 == 0, (sbw, d2)
    # chunk width for p-major pointwise loops
    cw = next(c for c in (512, 500, 400, 256, 200, 128, 100, 64, 48, 32, 20, 12, 8, 4)
              if pos % c == 0)
    nch = pos // cw
    assert sbw % cw == 0
    cwb = 2 * cw if pos % (2 * cw) == 0 else cw  # wide pointwise chunks
    nchb = pos // cwb
    # cn-apply / tr chunk: one sub-block (whole p-rows)
    rows_per = sbw // d2
    cwc = sbw
    nchc = pos // cwc
    tr_a, av_a, cc_a, out_a = alphas
    tr_bnz, av_bnz, cc_bnz = bias_flags
    KX = 65 if has_xbias else 64
    n_groups = n_cores // 4
    rgroups = [[g * 4 + j for j in range(4)] for g in range(n_groups)]

    nc = bacc.Bacc("TRN2", target_bir_lowering=False, debug=False,
                   num_devices=n_cores)

    x_in = nc.dram_tensor("x", [N, pos], f32, kind="ExternalInput")
    y_out = nc.dram_tensor("y", [OUT, pos], f32, kind="ExternalOutput")

    def din(name, shape, dt):
        return nc.dram_tensor(name, shape, dt, kind="ExternalInput")

    wt = {}
    for r in ("row", "col"):
        for i in range(layers):
            wt[f"whh_{r}{i}"] = din(f"whh_{r}{i}", [H, 2, 4, H], bf16)
            wt[f"wxb_{r}{i}"] = din(f"wxb_{r}{i}", [KX, 2, 4, H], bf16)
            wt[f"proj_{r}{i}"] = din(f"proj_{r}{i}", [H, 2, N], bf16)
            wt[f"pb_{r}{i}"] = din(f"pb_{r}{i}", [N, 1], f32)
    for i in range(layers):
        wt[f"trw{i}"] = din(f"trw{i}", [N, 3, H], f32r)
        wt[f"trb{i}"] = din(f"trb{i}", [H, 3], f32)
        wt[f"avw{i}"] = din(f"avw{i}", [H, 3, 3, H], bf16)
        wt[f"avb{i}"] = din(f"avb{i}", [H, 3], f32)
        wt[f"ccw{i}"] = din(f"ccw{i}", [H, 6, N], bf16)
        wt[f"ccb{i}"] = din(f"ccb{i}", [N, 1], f32)
        for nm in ("rn", "cn", "chn"):
            wt[f"{nm}w{i}"] = din(f"{nm}w{i}", [N, 1], f32)
            wt[f"{nm}b{i}"] = din(f"{nm}b{i}", [N, 1], f32)
    wt["outw"] = din("outw", [N, OUT], f32r)
    wt["outb"] = din("outb", [OUT, 1], f32)
    wt["msk"] = din("msk", [H, 1], f32)

    with tile.TileContext(nc) as tc:
        with contextlib.ExitStack() as ctx:
            singles = ctx.enter_context(tc.tile_pool(name="singles", bufs=1))
            xz = ctx.enter_context(tc.tile_pool(name="xz", bufs=2))
            ps = ctx.enter_context(tc.tile_pool(name="ps", bufs=2, space="PSUM"))
            pscan = ctx.enter_context(tc.tile_pool(name="pscan", bufs=1, space="PSUM"))
            tmp = ctx.enter_context(tc.tile_pool(name="tmp", bufs=2))
            chk = ctx.enter_context(tc.tile_pool(name="chk", bufs=2))
            dram = ctx.enter_context(tc.tile_pool(name="dram", bufs=1, space="DRAM"))

            sw = {}
            for k, t in wt.items():
                sw[k] = singles.tile(list(t.shape), t.dtype, tag=f"w_{k}",
                                     name=f"sw_{k}")

            def _wprio(k):
                li = int(k[-1]) if k[-1].isdigit() else layers
                kind = 0 if "row" in k else (1 if "col" in k else 2)
                return (li, kind, k)

            def load_weights():
                for k in sorted(wt.keys(), key=_wprio):
                    nc.sync.dma_start(out=sw[k][:], in_=wt[k][:])

            out_dram = dram.tile([N, pos], f32r)
            ch_dram = dram.tile([H, 3, pos], bf16)
            hf_dram = dram.tile([H, pos], bf16)
            hb_dram = dram.tile([H, pos], bf16)
            # sub-block bounce buffers: collectives pipelined per sub-block
            bounce_in_s = [dram.tile([4, 3, H, sbw], bf16, tag=f"bin{j}", name=f"bin{j}")
                           for j in range(n_sub)]
            bounce_rs_s = [dram.tile([3, H, sbw], bf16, tag=f"brs{j}", name=f"brs{j}")
                           for j in range(n_sub)]
            bounce_cm_s = [dram.tile([4, 3, H, sbw], bf16, tag=f"bcm{j}", name=f"bcm{j}")
                           for j in range(n_sub)]
            gn_dram = dram.tile([1, 2], f32)

            def new_x65():
                return xz.tile([KX, pos], bf16, tag="xz", name="x65t")

            def new_z():
                return xz.tile([N, pos], bf16, tag="xz", name="zt")

            ones_k = singles.tile([N, 1], f32, tag="ones_k")
            nc.vector.memset(ones_k[:], 1.0)
            epsc = singles.tile([1, 1], f32, tag="epsc")
            nc.vector.memset(epsc[:], 1e-8)

            def gn_finalize(stats, wv, bv):
                """stats [N, nchunks, 6] -> per-partition s,t [N,1] f32."""
                mv = tmp.tile([N, 2], f32, tag="gnmv")
                nc.vector.bn_aggr(out=mv[:], in_=stats[:])
                r3 = tmp.tile([N, 3], f32, tag="gnr3")
                nc.vector.tensor_copy(r3[:, 0:2], mv[:, 0:2])
                nc.vector.tensor_mul(r3[:, 2:3], mv[:, 0:1], mv[:, 0:1])
                pr = ps.tile([128, 512], f32, tag="ps")
                nc.tensor.matmul(pr[0:1, 0:3], ones_k[:], r3[:],
                                 start=True, stop=True)
                sc = tmp.tile([1, 3], f32, tag="gnsc")
                nc.vector.tensor_scalar_mul(sc[0:1, 0:3], pr[0:1, 0:3], 1.0 / N)
                var = tmp.tile([1, 1], f32, tag="gnvar")
                nc.vector.tensor_mul(var[0:1], sc[0:1, 0:1], sc[0:1, 0:1])
                nc.vector.tensor_sub(var[0:1], sc[0:1, 1:2], var[0:1])
                nc.vector.tensor_add(var[0:1], var[0:1], sc[0:1, 2:3])
                # wait: var = S1/N - mu^2 + S2/N  (S1=sum var_p, S2=sum mu_p^2)
                rm = tmp.tile([1, 2], f32, tag="gnrm")
                nc.scalar.activation(out=rm[0:1, 0:1], in_=var[0:1], func=AF.Sqrt,
                                     bias=epsc[0:1, :], scale=1.0)
                nc.vector.reciprocal(rm[0:1, 0:1], rm[0:1, 0:1])
                nc.vector.tensor_scalar_mul(rm[0:1, 1:2], sc[0:1, 0:1], -1.0)
                nc.sync.dma_start(out=gn_dram[0:1, :], in_=rm[0:1, :])
                rb = tmp.tile([N, 2], f32, tag="gnrb")
                bcast = bass.AP(tensor=gn_dram[:].tensor, offset=gn_dram[:].offset,
                                ap=[[0, N], [1, 2]])
                nc.sync.dma_start(out=rb[:], in_=bcast)
                s = tmp.tile([N, 1], f32, tag="gns")
                t = tmp.tile([N, 1], f32, tag="gnt")
                nc.vector.tensor_mul(s[:], wv[:], rb[:, 0:1])
                nc.vector.scalar_tensor_tensor(
                    out=t[:], in0=s[:], scalar=rb[:, 1:2], in1=bv[:],
                    op0=OP.mult, op1=OP.add)
                return s, t

            # ---------------- LSTM scan ----------------
            def scan(r, i, x65t, T, M, x_ap, hdst):
                """Software-pipelined scan over phase-shifted streams
                (direction x column-range). Round-robin emission lets the
                scheduler overlap one stream's ACT/DVE chain with another
                stream's matmuls. x_ap(d, k, c0, Ms) -> AP [KX, Ms];
                hdst(d, seq, c0, Ms) -> DRAM AP [H, Ms]."""
                whh = sw[f"whh_{r}{i}"]
                wxb = sw[f"wxb_{r}{i}"]
                nh = max(1, n_streams // 2)
                csz = M // nh
                assert csz * nh == M
                streams = [(d, h * csz) for d in range(2) for h in range(nh)]
                GS = 128 if csz <= 128 else 256
                st = []
                for si, (d, c0) in enumerate(streams):
                    wst = tmp.tile([H, csz], f32, tag=f"wst{si}",
                                   name=f"wst{si}")
                    nc.vector.memset(wst[:], 0.0)
                    hr = tmp.tile([H, 2, csz], bf16, tag=f"hr{si}",
                                  name=f"hr{si}")
                    st.append((wst, hr))
                for k in range(T):
                    for si, (d, c0) in enumerate(streams):
                        wst, hr = st[si]
                        seq = k if d == 0 else (T - 1 - k)
                        pst = pscan.tile([128, 4, GS], f32, tag=f"pss{si}",
                                         name=f"pss{si}")
                        for g in range(4):
                            # start=True clears has_written for the WHOLE
                            # bank; only the first matmul into each bank may
                            # set it (later gates in the bank find cleared
                            # bits and overwrite, then hh accumulates).
                            first_in_bank = (
                                g == 0 or (g * GS) // 512 != ((g - 1) * GS) // 512)
                            nc.tensor.matmul(
                                pst[:, g, 0:csz], wxb[:, d, g, :],
                                x_ap(d, k, c0, csz),
                                start=first_in_bank, stop=(k == 0))
                        if k > 0:
                            for g in range(4):
                                nc.tensor.matmul(
                                    pst[:, g, 0:csz], whh[:, d, g, :],
                                    hr[:, (k - 1) % 2, :],
                                    start=False, stop=True)
                        Tt = tmp.tile([H, 4, csz], f32, tag=f"Tt{si}",
                                      name=f"Tt{si}")
                        nc.scalar.activation(out=Tt[:], in_=pst[:, :, 0:csz],
                                             func=AF.Tanh)
                        pt = tmp.tile([H, csz], f32, tag=f"pt{si}",
                                      name=f"pt{si}")
                        qt = tmp.tile([H, csz], f32, tag=f"qt{si}",
                                      name=f"qt{si}")
                        nc.vector.scalar_tensor_tensor(
                            out=qt[:], in0=Tt[:, 0, :], scalar=1.0,
                            in1=Tt[:, 2, :], op0=OP.add, op1=OP.mult)
                        nc.vector.scalar_tensor_tensor(
                            out=pt[:], in0=Tt[:, 1, :], scalar=1.0,
                            in1=wst[:], op0=OP.add, op1=OP.mult)
                        nc.vector.scalar_tensor_tensor(
                            out=wst[:], in0=pt[:], scalar=0.5,
                            in1=qt[:], op0=OP.mult, op1=OP.add)
                        Tc = tmp.tile([H, csz], f32, tag=f"Tc{si}",
                                      name=f"Tc{si}")
                        nc.scalar.activation(out=Tc[:], in_=wst[:],
                                             func=AF.Tanh, scale=0.5)
                        nc.vector.scalar_tensor_tensor(
                            out=hr[:, k % 2, :], in0=Tt[:, 3, :],
                            scalar=1.0, in1=Tc[:], op0=OP.add, op1=OP.mult)
                        nc.sync.dma_start(out=hdst(d, seq, c0, csz),
                                          in_=hr[:, k % 2, :])

            # ------------- proj + GN stats -------------
            def proj_phase(r, i, nchunks, cwp, gnw, gnb):
                """Ĥ streamed from HBM in contiguous cwp chunks."""
                z = new_z()
                pj = sw[f"proj_{r}{i}"]
                pb = sw[f"pb_{r}{i}"]
                stats = tmp.tile([N, nchunks, 6], f32, tag="gnstats")
                for ci in range(nchunks):
                    off = ci * cwp
                    hfc = chk.tile([H, cwp], bf16, tag="hfc")
                    hbc = chk.tile([H, cwp], bf16, tag="hbc")
                    nc.sync.dma_start(out=hfc[:], in_=hf_dram[:, off:off + cwp])
                    nc.sync.dma_start(out=hbc[:], in_=hb_dram[:, off:off + cwp])
                    pp = ps.tile([128, 512], f32, tag="ps")
                    nc.tensor.matmul(pp[0:N, 0:cwp], pj[:, 0, :], hfc[:],
                                     start=True, stop=False)
                    nc.tensor.matmul(pp[0:N, 0:cwp], pj[:, 1, :], hbc[:],
                                     start=False, stop=True)
                    nc.vector.tensor_scalar_add(z[:, off:off + cwp],
                                                pp[0:N, 0:cwp], pb[:])
                    nc.vector.bn_stats(out=stats[:, ci, :], in_=z[:, off:off + cwp])
                s, t = gn_finalize(stats, gnw, gnb)
                return z, s, t

            # ---- chunked apply: out' = out + z*s + t, fused consumer ----
            def apply_chunks(z, s, t, consumer, cwa, ncha, z_ap=None,
                             first_src=None):
                for ci in range(ncha):
                    off = ci * cwa
                    outc = chk.tile([N, cwa], f32r, tag="outc")
                    src = first_src if first_src is not None else out_dram
                    nc.sync.dma_start(out=outc[:],
                                      in_=src[:, off:off + cwa].bitcast(f32r))
                    if z is not None:
                        v = chk.tile([N, cwa], bf16, tag="vt")
                        zin = z_ap(z, ci) if z_ap is not None else z[:, off:off + cwa]
                        nc.vector.tensor_scalar(
                            out=v[:], in0=zin, scalar1=s[:], scalar2=t[:],
                            op0=OP.mult, op1=OP.add)
                        nc.vector.tensor_add(outc[:], outc[:], v[:])
                        nc.sync.dma_start(out=out_dram[:, off:off + cwa],
                                          in_=outc[:])
                    consumer(ci, off, outc)

            def cast_consumer(x65t):
                def f(ci, off, outc):
                    nc.scalar.copy(out=x65t[0:N, off:off + outc.shape[1]],
                                   in_=outc[:])
                return f

            def dma_split_blk(dst4, sb, off, width, write=True):
                """DMA [H,3,width] SBUF <-> bounce[4,3,H,blk], splitting at
                block boundaries."""
                done = 0
                while done < width:
                    o = off + done
                    q, qo = o // blk, o % blk
                    wseg = min(width - done, blk - qo)
                    dr = dst4[q, :, :, qo:qo + wseg].transpose([1, 0, 2])
                    if write:
                        nc.sync.dma_start(out=dr, in_=sb[:, :, done:done + wseg])
                    else:
                        nc.sync.dma_start(out=sb[:, :, done:done + wseg], in_=dr)
                    done += wseg

            # ================= network =================
            def _network():
                x65 = new_x65()
                if has_xbias:
                    nc.vector.memset(x65[N:KX, :], 1.0)

                def init_consumer(x65t):
                    def f(ci, off, outc):
                        nc.sync.dma_start(out=out_dram[:, off:off + cwb],
                                          in_=outc[:])
                        nc.scalar.copy(out=x65t[0:N, off:off + cwb], in_=outc[:])
                    return f

                apply_chunks(None, None, None, init_consumer(x65), cwb, nchb,
                             first_src=x_in)
                load_weights()

                for i in range(layers):
                    # ---- row: sequences along d1 (t=p), batch d2 ----
                    scan("row", i, x65, d1, d2,
                         lambda d, k, c0, Ms: x65[
                             0:KX,
                             (k if d == 0 else d1 - 1 - k) * d2 + c0:
                             (k if d == 0 else d1 - 1 - k) * d2 + c0 + Ms],
                         lambda d, seq, c0, Ms: (hf_dram if d == 0 else hb_dram)[
                             :, seq * d2 + c0:seq * d2 + c0 + Ms])
                    z, s, t = proj_phase("row", i, nch, cw, sw[f"rnw{i}"], sw[f"rnb{i}"])
                    x65c = new_x65()
                    if has_xbias:
                        nc.vector.memset(x65c[N:KX, :], 1.0)
                    apply_chunks(z, s, t, cast_consumer(x65c), cwb, nchb)

                    # ---- col: sequences along d2 (t=q), batch d1; strided x ----
                    def xcol(d, k, c0, Ms, _x=x65c):
                        q = k if d == 0 else d2 - 1 - k
                        a = _x[0:KX, q + c0 * d2:q + c0 * d2 + 1]
                        return bass.AP(tensor=a.tensor, offset=a.offset,
                                       ap=[a.ap[0], [d2, Ms]])

                    scan("col", i, x65c, d2, d1, xcol,
                         lambda d, seq, c0, Ms: (hf_dram if d == 0 else hb_dram)[
                             :, seq * d1 + c0:seq * d1 + c0 + Ms])
                    # col proj in q-major contiguous chunks; z stored q-major
                    zq, s2_, t2_ = proj_phase("col", i, nch, cw,
                                              sw[f"cnw{i}"], sw[f"cnb{i}"])

                    # cn-apply in p-row-aligned chunks with strided z view,
                    # fused with TAC tr + mask + bounce write
                    def zq_ap(zt, ci, _rp=rows_per):
                        p0 = ci * _rp
                        a = zt[:, p0:p0 + 1]
                        return bass.AP(tensor=a.tensor, offset=a.offset,
                                       ap=[a.ap[0], [1, _rp], [d1, d2]])

                    trw = sw[f"trw{i}"]
                    trb = sw[f"trb{i}"]
                    msk = sw["msk"]
                    nmm = (cwc + 511) // 512  # matmul pieces per tr chunk
                    pw = cwc // nmm

                    def tr_consumer(ci, off, outc, _i=i):
                        q, jj = ci // n_sub, ci % n_sub
                        chc = chk.tile([H, 3, cwc], bf16, tag="chc")
                        for e in range(3):
                            pp2 = pscan.tile([128, 2, 512], f32, tag="pstr")
                            for h2 in range(nmm):
                                nc.tensor.matmul(
                                    pp2[:, h2, 0:pw], trw[:, e, :],
                                    outc[:, h2 * pw:(h2 + 1) * pw],
                                    start=True, stop=True)
                            if tr_bnz:
                                nc.vector.tensor_scalar_add(
                                    pp2[:, 0:nmm, 0:pw], pp2[:, 0:nmm, 0:pw],
                                    trb[:, e:e + 1])
                            rl = chk.tile([H, cwc], bf16, tag="rl")
                            nc.scalar.activation(
                                out=rl[:], in_=pp2[:, 0:nmm, 0:pw],
                                func=AF.Relu, scale=1.0 - tr_a[_i])
                            nc.vector.scalar_tensor_tensor(
                                out=chc[:, e, :], in0=pp2[:, 0:nmm, 0:pw],
                                scalar=tr_a[_i], in1=rl[:],
                                op0=OP.mult, op1=OP.add)
                        nc.sync.dma_start(out=ch_dram[:, :, off:off + cwc],
                                          in_=chc[:])
                        con = chk.tile([H, 3, cwc], bf16, tag="conc")
                        nc.vector.tensor_scalar_mul(con[:], chc[:], msk[:])
                        nc.sync.dma_start(
                            out=bounce_in_s[jj][q].transpose([1, 0, 2]),
                            in_=con[:])

                    # ---- tr / ReduceScatter / av / AllGather pipelined
                    # per sub-block: chunks ordered so each sub-block's 4
                    # quarters finish together, then its collective runs
                    # while the next sub-block computes.
                    avw = sw[f"avw{i}"]
                    avb = sw[f"avb{i}"]
                    bw = next(c for c in (500, 512, 400, 256, 250, 200, 128,
                                          100, 64, 50, 40, 25, 20, 10, 5, 4,
                                          2, 1)
                              if sbw % c == 0)
                    for jj in range(n_sub):
                        for q in range(4):
                            ci = q * n_sub + jj
                            off = ci * cwc
                            outc = chk.tile([N, cwc], f32r, tag="outc")
                            nc.sync.dma_start(
                                out=outc[:],
                                in_=out_dram[:, off:off + cwc].bitcast(f32r))
                            v = chk.tile([N, cwc], bf16, tag="vt")
                            nc.vector.tensor_scalar(
                                out=v[:], in0=zq_ap(zq, ci), scalar1=s2_[:],
                                scalar2=t2_[:], op0=OP.mult, op1=OP.add)
                            nc.vector.tensor_add(outc[:], outc[:], v[:])
                            nc.sync.dma_start(out=out_dram[:, off:off + cwc],
                                              in_=outc[:])
                            tr_consumer(ci, off, outc)
                        if with_cc:
                            nc.gpsimd.collective_compute(
                                "ReduceScatter", OP.add,
                                replica_groups=rgroups,
                                ins=[bounce_in_s[jj].opt()],
                                outs=[bounce_rs_s[jj].opt()])
                        # av on this sub-block of the local quarter
                        for bo in range(0, sbw, bw):
                            cmp_ = chk.tile([H, 3, bw], bf16, tag="cmp")
                            nc.sync.dma_start(
                                out=cmp_[:],
                                in_=bounce_rs_s[jj][:, :, bo:bo + bw]
                                .transpose([1, 0, 2]))
                            cml = chk.tile([H, 3, bw], bf16, tag="cml")
                            for mt in range(3):
                                pp = ps.tile([128, 512], f32, tag="ps")
                                for kt in range(3):
                                    nc.tensor.matmul(
                                        pp[:, 0:bw], avw[:, kt, mt, :],
                                        cmp_[:, kt, :], start=(kt == 0),
                                        stop=(kt == 2))
                                if av_bnz:
                                    nc.vector.tensor_scalar_add(
                                        pp[:, 0:bw], pp[:, 0:bw],
                                        avb[:, mt:mt + 1])
                                rl2 = chk.tile([H, bw], f32, tag="rl", name="rl2")
                                nc.scalar.activation(
                                    out=rl2[:], in_=pp[:, 0:bw],
                                    func=AF.Relu, scale=1.0 - av_a[i])
                                nc.vector.scalar_tensor_tensor(
                                    out=cml[:, mt, :], in0=pp[:, 0:bw],
                                    scalar=av_a[i], in1=rl2[:],
                                    op0=OP.mult, op1=OP.add)
                            nc.sync.dma_start(
                                out=bounce_rs_s[jj][:, :, bo:bo + bw]
                                .transpose([1, 0, 2]),
                                in_=cml[:])
                        if with_cc:
                            nc.gpsimd.collective_compute(
                                "AllGather", OP.bypass,
                                replica_groups=rgroups,
                                ins=[bounce_rs_s[jj].opt()],
                                outs=[bounce_cm_s[jj].opt()])

                    # ---- cc + chn stats (sub-block order: consume each
                    # sub-block's cm as its AllGather lands) ----
                    ccw = sw[f"ccw{i}"]
                    ccb = sw[f"ccb{i}"]
                    z2 = new_z()
                    stats2 = tmp.tile([N, nch, 6], f32, tag="gnstats")
                    hpc = sbw // cw
                    for jj in range(n_sub):
                        for q in range(4):
                            for h2 in range(hpc):
                                ci = q * (nch // 4) + jj * hpc + h2
                                off = ci * cw
                                chc = chk.tile([H, 3, cw], bf16, tag="chc", name="chc_cc")
                                nc.sync.dma_start(
                                    out=chc[:],
                                    in_=ch_dram[:, :, off:off + cw])
                                cmc = chk.tile([H, 3, cw], bf16, tag="conc", name="cmc")
                                nc.sync.dma_start(
                                    out=cmc[:],
                                    in_=bounce_cm_s[jj][
                                        q, :, :, h2 * cw:(h2 + 1) * cw]
                                    .transpose([1, 0, 2]))
                                pp = ps.tile([128, 512], f32, tag="ps")
                                for kt in range(3):
                                    nc.tensor.matmul(
                                        pp[0:N, 0:cw], ccw[:, kt, :],
                                        chc[:, kt, :], start=(kt == 0),
                                        stop=False)
                                for kt in range(3):
                                    nc.tensor.matmul(
                                        pp[0:N, 0:cw], ccw[:, 3 + kt, :],
                                        cmc[:, kt, :], start=False,
                                        stop=(kt == 2))
                                if cc_bnz:
                                    nc.vector.tensor_scalar_add(
                                        pp[0:N, 0:cw], pp[0:N, 0:cw], ccb[:])
                                rl3 = chk.tile([N, cw], f32, tag="rl", name="rl3")
                                nc.scalar.activation(
                                    out=rl3[:], in_=pp[0:N, 0:cw],
                                    func=AF.Relu, scale=1.0 - cc_a[i])
                                nc.vector.scalar_tensor_tensor(
                                    out=z2[:, off:off + cw],
                                    in0=pp[0:N, 0:cw], scalar=cc_a[i],
                                    in1=rl3[:], op0=OP.mult, op1=OP.add)
                                nc.vector.bn_stats(out=stats2[:, ci, :],
                                                   in_=z2[:, off:off + cw])
                    s2, t2 = gn_finalize(stats2, sw[f"chnw{i}"], sw[f"chnb{i}"])

                    if i < layers - 1:
                        x65 = new_x65()
                        if has_xbias:
                            nc.vector.memset(x65[N:KX, :], 1.0)
                        apply_chunks(z2, s2, t2, cast_consumer(x65), cwb, nchb)
                    else:
                        outw = sw["outw"]
                        outb = sw["outb"]

                        def fin_consumer(ci, off, outc):
                            rl4 = chk.tile([N, cw], f32, tag="rl", name="rl4")
                            nc.scalar.activation(out=rl4[:], in_=outc[:],
                                                 func=AF.Relu, scale=1.0 - out_a)
                            yp = chk.tile([N, cw], f32r, tag="yp")
                            nc.vector.scalar_tensor_tensor(
                                out=yp[:], in0=outc[:], scalar=out_a,
                                in1=rl4[:], op0=OP.mult, op1=OP.add)
                            pp = ps.tile([128, 512], f32, tag="ps")
                            nc.tensor.matmul(pp[0:OUT, 0:cw], outw[:], yp[:],
                                             start=True, stop=True)
                            yc = chk.tile([OUT, cw], f32, tag="yc")
                            nc.vector.tensor_scalar_add(yc[:], pp[0:OUT, 0:cw],
                                                        outb[:])
                            nc.sync.dma_start(out=y_out[:, off:off + cw], in_=yc[:])

                        apply_chunks(z2, s2, t2, fin_consumer, cw, nch)


            if n_iter == 1:
                _network()
            else:
                with tc.For_i(0, n_iter, 1):
                    _network()

    nc.compile()
    return nc


def make_in_maps(inputs, d1=D1, d2=D2, layers=L):
    x = np.asarray(inputs["x"], np.float32)
    per_core = []
    has_xbias = None
    for b in range(B):
        for c in range(CH):
            w = prep_weights(inputs, b, c, layers)
            has_xbias = w.pop("_has_xbias")
            m = {"x": np.ascontiguousarray(x[b, c].reshape(N, d1 * d2))}
            for k, v in w.items():
                m[k] = np.ascontiguousarray(v)
            per_core.append(m)
    alphas = (
        [float(np.asarray(inputs["tr_a"][i])) for i in range(layers)],
        [float(np.asarray(inputs["av_a"][i])) for i in range(layers)],
        [float(np.asarray(inputs["cc_a"][i])) for i in range(layers)],
        float(np.asarray(inputs["out_a"])),
    )
    bias_flags = tuple(
        bool(np.any(np.asarray(inputs[k]) != 0))
        for k in ("tr_b", "av_b", "cc_b"))
    return per_core, has_xbias, alphas, bias_flags


def kernel(**inputs):
    import os
    import concourse.bass_utils as bass_utils

    n_streams = int(os.environ.get("BASS_NSTREAM", "2"))
    per_core, has_xbias, alphas, bias_flags = make_in_maps(inputs)
    ck = (has_xbias, tuple(map(tuple, alphas[:3])), alphas[3], bias_flags,
          n_streams)
    if ck not in _CACHE:
        _CACHE[ck] = build_program(has_xbias, alphas, bias_flags,
                                   n_streams=n_streams)
    nc = _CACHE[ck]
    r = bass_utils.run_bass_kernel_spmd(nc, per_core, core_ids=list(range(NCORES)))
    ys = np.zeros((B * CH, OUT, D1, D2), np.float32)
    for ci in range(NCORES):
        ys[ci] = r.results[ci]["y"].reshape(OUT, D1, D2)
    return ys

